# revision 6
# baseline (speedup 1.0000x reference)
"""DTCWT inverse (qshift, single level) as a Bass/Tile kernel for TRN2.

Factorization (column filter first):
    out = (C0·Yl + C1·lh)·C0^T + (C0·hl + C1·hh)·C1^T
with C0/C1 the 256x128 banded synthesis (colifilt) matrices; the rowifilt
matrices are identical, so the same SBUF statics serve both stages.

Per slice (128x128 images), two PE stages of 4 matmuls each, all with
256 moving rows:
    stage 1:  z1^T = Yl^T-contract:  z1^T[c,i] = sum_r Yl[r,c]·C0T[r,i] + ...
              A = image (stationary, natural layout), B = C*T (moving)
              -> psum [128c, 512] = [z1^T | z2^T]
    stage 2:  out[i-tile] = zs-half^T @ C*T
              A = zs[:, tile] (stationary), B = C0T/C1T (moving)
              -> psum [128i, 512] = [rows 0:128 | rows 128:256]

Everything is bf16 (inputs, statics, mid-stage, output) with f32 PSUM
accumulation; validated rel err ~5.6e-3 vs the f32 reference.

c2q quadrants: quad = QA + QB where QA/QB are host-packed per-band images
(column-interleaved, signs baked in); one fused DVE add per group. The
1/sqrt(2) c2q scale is folded into the C0s/C1s statics used by stage 1
for the quadrant terms.

Host packs inputs partition-major so every DMA descriptor is a >=2KB
contiguous line; output rows are 512B contiguous bf16 writes.
"""
import numpy as np
import ml_dtypes

import concourse.bacc as bacc
import concourse.tile as tile
from concourse import mybir

F32 = mybir.dt.float32
BF16 = mybir.dt.bfloat16
NPBF16 = ml_dtypes.bfloat16

# quad -> (band1, band2): hl, lh, hh.  quad row filters: hl,lh,hh use
# (C0s, C1s, C1s) in stage 1 and pair with (C0, C0, C1)... see stage map.
QUAD_BANDS = ((2, 3), (0, 5), (1, 4))   # hl, lh, hh

# ---------------- host-side static matrix construction ----------------

_H0A = np.array([0.0351638365171441, 0.0, -0.0883294244510729,
                 0.233890320607236, 0.760272369066126, 0.587518297723561,
                 0.0, -0.114301837144249, 0.0, 0.0], dtype=np.float64)
_H0B = _H0A[::-1].copy()
_ALT = (-1.0) ** np.arange(10)
_H1A = _H0B * _ALT
_H1B = _H1A[::-1].copy()
G0A, G0B, G1A, G1B = _H0B, _H0A, _H1B, _H1A


def _reflect(x, minx, maxx):
    x = np.asarray(x, dtype=np.float64)
    rng = maxx - minx
    rng2 = 2.0 * rng
    mod = np.fmod(x - minx, rng2)
    normed = np.where(mod < 0, mod + rng2, mod)
    return (np.where(normed >= rng, rng2 - normed, normed) + minx).astype(np.int64)


def _colifilt_matrix(ha, hb, r=128):
    """C (2r x r) with colifilt(X) = C @ X."""
    m = ha.shape[0]
    m2 = m // 2
    xe = _reflect(np.arange(-m2, r + m2), -0.5, r - 0.5)
    t = np.arange(2, r + m - 1, 2)
    if float(np.sum(ha * hb)) > 0:
        ta, tb = t, t - 1
    else:
        ta, tb = t - 1, t
    r2 = r // 2
    hao, hae = ha[0::2], ha[1::2]
    hbo, hbe = hb[0::2], hb[1::2]

    def vconv_mat(sel_idx, h):
        hf = h[::-1]
        M = np.zeros((r2, r), dtype=np.float64)
        for i in range(r2):
            for k in range(m2):
                M[i, sel_idx[i + k]] += hf[k]
        return M

    C = np.zeros((2 * r, r), dtype=np.float64)
    C[0::4] = vconv_mat(xe[tb], hao)
    C[1::4] = vconv_mat(xe[ta], hbo)
    C[2::4] = vconv_mat(xe[tb], hae)
    C[3::4] = vconv_mat(xe[ta], hbe)
    return C


def build_statics():
    """CT [128, 1024] bf16 = [C0T | C1T | C0sT | C1sT], s = 1/sqrt(2)."""
    C0 = _colifilt_matrix(G0B, G0A)
    C1 = _colifilt_matrix(G1B, G1A)
    s = 1.0 / np.sqrt(2.0)
    CT = np.concatenate([C0.T, C1.T, (s * C0).T, (s * C1).T], axis=1)
    return np.ascontiguousarray(CT.astype(np.float32).astype(NPBF16))


# ---------------- device kernel ----------------


def build_kernel(n_ch=64, G=4, n_cores=8):
    nc = bacc.Bacc("TRN2", target_bir_lowering=False, debug=False,
                   num_devices=n_cores)
    YLT = nc.dram_tensor("YLT", [128, n_ch, 128], BF16, kind="ExternalInput").ap()
    QA = nc.dram_tensor("QA", [3, 128, n_ch, 128], BF16, kind="ExternalInput").ap()
    QB = nc.dram_tensor("QB", [3, 128, n_ch, 128], BF16, kind="ExternalInput").ap()
    CTD = nc.dram_tensor("CT", [128, 1024], BF16, kind="ExternalInput").ap()
    OUT = nc.dram_tensor("Y", [n_ch, 256, 256], BF16, kind="ExternalOutput").ap()

    assert n_ch % G == 0 and G % 2 == 0
    with tile.TileContext(nc) as tc:
        with (
            tc.tile_pool(name="const", bufs=1) as const,
            tc.tile_pool(name="inp", bufs=2) as inp,
            tc.tile_pool(name="quad", bufs=2) as quad,
            tc.tile_pool(name="zt", bufs=3) as ztp,
            tc.tile_pool(name="yout", bufs=2) as yp,
            tc.tile_pool(name="psz", bufs=2, space="PSUM") as pp,
            tc.tile_pool(name="psy", bufs=2, space="PSUM") as ppy,
        ):
            ct = const.tile([128, 1024], BF16)
            nc.sync.dma_start(ct[:], CTD[:])
            C0T = ct[:, 0:256]
            C1T = ct[:, 256:512]
            C0sT = ct[:, 512:768]
            C1sT = ct[:, 768:1024]

            def load_group(g0):
                TL = inp.tile([128, G * 128], BF16, tag="TL")
                nc.sync.dma_start(
                    TL.rearrange("p (g c) -> p g c", g=G),
                    YLT[:, g0:g0 + G],
                )
                qa = inp.tile([128, 3 * G * 128], BF16, tag="qa")
                qb = inp.tile([128, 3 * G * 128], BF16, tag="qb")
                nc.sync.dma_start(
                    qa.rearrange("p (q g c) -> p q g c", q=3, g=G),
                    QA[:, :, g0:g0 + G].rearrange("q p g c -> p q g c"),
                )
                nc.sync.dma_start(
                    qb.rearrange("p (q g c) -> p q g c", q=3, g=G),
                    QB[:, :, g0:g0 + G].rearrange("q p g c -> p q g c"),
                )
                return TL, qa, qb

            def prep_group(state):
                TL, qa, qb = state
                QD = quad.tile([128, 3 * G * 128], BF16, tag="qd")
                nc.vector.tensor_add(QD[:], qa[:], qb[:])
                return TL, QD

            def process_group(g0, state):
                TL, QD = state
                qv = QD.rearrange("p (q g c) -> p q g c", q=3, g=G)
                YB = yp.tile([128, G * 512], BF16, tag="yb")
                for pr in range(G // 2):
                    # two slices per PSUM tile (2 banks) so the psum->sbuf
                    # copies amortize their fixed per-op overhead
                    zp = pp.tile([128, 1024], F32, tag="zp")
                    for j in range(2):
                        ci = pr * 2 + j
                        cs = slice(ci * 128, (ci + 1) * 128)
                        o = j * 512
                        zb = zp[:, o:o + 512]
                        nc.tensor.matmul(zb[:, 0:256], TL[:, cs], C0T,
                                         start=True, stop=False, skip_group_check=True)
                        nc.tensor.matmul(zb[:, 0:256], qv[:, 1, ci], C1sT,
                                         start=False, stop=False, skip_group_check=True)
                        nc.tensor.matmul(zb[:, 256:512], qv[:, 0, ci], C0sT,
                                         start=False, stop=False, skip_group_check=True)
                        nc.tensor.matmul(zb[:, 256:512], qv[:, 2, ci], C1sT,
                                         start=False, stop=True, skip_group_check=True)

                    zs = ztp.tile([128, 1024], BF16, tag="zs")
                    nc.scalar.copy(zs[:], zp[:])

                    op = ppy.tile([128, 1024], F32, tag="op")
                    for j in range(2):
                        o = j * 512
                        ob = op[:, o:o + 512]
                        nc.tensor.matmul(ob[:, 0:256], zs[:, o:o + 128], C0T,
                                         start=True, stop=False, skip_group_check=True)
                        nc.tensor.matmul(ob[:, 0:256], zs[:, o + 256:o + 384], C1T,
                                         start=False, stop=False, skip_group_check=True)
                        nc.tensor.matmul(ob[:, 256:512], zs[:, o + 128:o + 256], C0T,
                                         start=False, stop=False, skip_group_check=True)
                        nc.tensor.matmul(ob[:, 256:512], zs[:, o + 384:o + 512], C1T,
                                         start=False, stop=True, skip_group_check=True)

                    nc.vector.tensor_copy(
                        YB[:, pr * 1024:(pr + 1) * 1024], op[:])

                nc.sync.dma_start(
                    OUT[g0:g0 + G].rearrange("g (it p) w -> p g it w", it=2),
                    YB.rearrange("p (g it w) -> p g it w", g=G, it=2),
                )

            # software pipeline: load + prep g+1 before computing g
            groups = list(range(0, n_ch, G))
            state = prep_group(load_group(groups[0]))
            for idx, g0 in enumerate(groups):
                if idx + 1 < len(groups):
                    nxt = prep_group(load_group(groups[idx + 1]))
                    process_group(g0, state)
                    state = nxt
                else:
                    process_group(g0, state)

    nc.compile()
    return nc


# ---------------- host wrapper: shard, run on 8 cores, gather ----------------

_CACHED = {}


def _get_compiled():
    if "nc" not in _CACHED:
        _CACHED["nc"] = build_kernel(n_ch=64, G=4, n_cores=8)
        _CACHED["ct"] = build_statics()
    return _CACHED["nc"], _CACHED["ct"]


def _make_in_maps(Yl, Yhr, Yhi, CT):
    """Per-core input packing (pure layout: transpose/interleave/sign)."""
    B = Yl.shape[0]
    # YLT: [B, 128(r), C, 128(c)]
    YLT = np.ascontiguousarray(
        Yl.transpose(0, 2, 1, 3)).astype(NPBF16)
    QA = np.zeros((B, 3, 128, 64, 128), dtype=np.float32)
    QB = np.zeros((B, 3, 128, 64, 128), dtype=np.float32)
    for q, (b1, b2) in enumerate(QUAD_BANDS):
        r1 = Yhr[:, :, b1].transpose(0, 2, 1, 3)   # [B, h, C, w]
        i1 = Yhi[:, :, b1].transpose(0, 2, 1, 3)
        r2 = Yhr[:, :, b2].transpose(0, 2, 1, 3)
        i2 = Yhi[:, :, b2].transpose(0, 2, 1, 3)
        QA[:, q, 0::2, :, 0::2] = r1
        QA[:, q, 0::2, :, 1::2] = i1
        QA[:, q, 1::2, :, 0::2] = i1
        QA[:, q, 1::2, :, 1::2] = -r1
        QB[:, q, 0::2, :, 0::2] = r2
        QB[:, q, 0::2, :, 1::2] = i2
        QB[:, q, 1::2, :, 0::2] = -i2
        QB[:, q, 1::2, :, 1::2] = r2
    QA = QA.astype(NPBF16)
    QB = QB.astype(NPBF16)
    in_maps = []
    for b in range(B):
        in_maps.append({
            "YLT": np.ascontiguousarray(YLT[b]),
            "QA": np.ascontiguousarray(QA[b]),
            "QB": np.ascontiguousarray(QB[b]),
            "CT": CT,
        })
    return in_maps


def kernel(Yl, Yhr, Yhi):
    """Inverse DTCWT (qshift) level. Yl (8,64,128,128) f32,
    Yhr/Yhi (8,64,6,64,64) f32 -> (8,64,256,256) f32.
    Data-parallel over the batch dim: one batch element per NeuronCore."""
    from concourse.bass_utils import run_bass_kernel_spmd

    Yl = np.asarray(Yl, dtype=np.float32)
    Yhr = np.asarray(Yhr, dtype=np.float32)
    Yhi = np.asarray(Yhi, dtype=np.float32)
    B = Yl.shape[0]
    assert B == 8, f"expected batch 8, got {B}"

    nc, CT = _get_compiled()
    in_maps = _make_in_maps(Yl, Yhr, Yhi, CT)
    res = run_bass_kernel_spmd(nc, in_maps, core_ids=list(range(B)))
    out = np.stack([np.asarray(res.results[b]["Y"]) for b in range(B)])
    return out.astype(np.float32)


# revision 8
# speedup vs baseline: 1.1133x; 1.1133x over previous
"""DTCWT inverse (qshift, single level) as a Bass/Tile kernel for TRN2.

Factorization (column filter first):
    out = (C0·Yl + C1·lh)·C0^T + (C0·hl + C1·hh)·C1^T
with C0/C1 the 256x128 banded synthesis (colifilt) matrices; the rowifilt
matrices are identical, so the same SBUF statics serve both stages.

Per slice (128x128 images), two PE stages of 4 matmuls each, all with
256 moving rows:
    stage 1:  z1^T = Yl^T-contract:  z1^T[c,i] = sum_r Yl[r,c]·C0T[r,i] + ...
              A = image (stationary, natural layout), B = C*T (moving)
              -> psum [128c, 512] = [z1^T | z2^T]
    stage 2:  out[i-tile] = zs-half^T @ C*T
              A = zs[:, tile] (stationary), B = C0T/C1T (moving)
              -> psum [128i, 512] = [rows 0:128 | rows 128:256]

Everything is bf16 (inputs, statics, mid-stage, output) with f32 PSUM
accumulation; validated rel err ~5.6e-3 vs the f32 reference.

c2q quadrants: quad = QA + QB where QA/QB are host-packed per-band images
(column-interleaved, signs baked in); one fused DVE add per group. The
1/sqrt(2) c2q scale is folded into the C0s/C1s statics used by stage 1
for the quadrant terms.

Host packs inputs partition-major so every DMA descriptor is a >=2KB
contiguous line; output rows are 512B contiguous bf16 writes.
"""
import numpy as np
import ml_dtypes

import concourse.bacc as bacc
import concourse.tile as tile
from concourse import mybir

F32 = mybir.dt.float32
BF16 = mybir.dt.bfloat16
NPBF16 = ml_dtypes.bfloat16

# quad -> (band1, band2): hl, lh, hh.  quad row filters: hl,lh,hh use
# (C0s, C1s, C1s) in stage 1 and pair with (C0, C0, C1)... see stage map.
QUAD_BANDS = ((2, 3), (0, 5), (1, 4))   # hl, lh, hh

# ---------------- host-side static matrix construction ----------------

_H0A = np.array([0.0351638365171441, 0.0, -0.0883294244510729,
                 0.233890320607236, 0.760272369066126, 0.587518297723561,
                 0.0, -0.114301837144249, 0.0, 0.0], dtype=np.float64)
_H0B = _H0A[::-1].copy()
_ALT = (-1.0) ** np.arange(10)
_H1A = _H0B * _ALT
_H1B = _H1A[::-1].copy()
G0A, G0B, G1A, G1B = _H0B, _H0A, _H1B, _H1A


def _reflect(x, minx, maxx):
    x = np.asarray(x, dtype=np.float64)
    rng = maxx - minx
    rng2 = 2.0 * rng
    mod = np.fmod(x - minx, rng2)
    normed = np.where(mod < 0, mod + rng2, mod)
    return (np.where(normed >= rng, rng2 - normed, normed) + minx).astype(np.int64)


def _colifilt_matrix(ha, hb, r=128):
    """C (2r x r) with colifilt(X) = C @ X."""
    m = ha.shape[0]
    m2 = m // 2
    xe = _reflect(np.arange(-m2, r + m2), -0.5, r - 0.5)
    t = np.arange(2, r + m - 1, 2)
    if float(np.sum(ha * hb)) > 0:
        ta, tb = t, t - 1
    else:
        ta, tb = t - 1, t
    r2 = r // 2
    hao, hae = ha[0::2], ha[1::2]
    hbo, hbe = hb[0::2], hb[1::2]

    def vconv_mat(sel_idx, h):
        hf = h[::-1]
        M = np.zeros((r2, r), dtype=np.float64)
        for i in range(r2):
            for k in range(m2):
                M[i, sel_idx[i + k]] += hf[k]
        return M

    C = np.zeros((2 * r, r), dtype=np.float64)
    C[0::4] = vconv_mat(xe[tb], hao)
    C[1::4] = vconv_mat(xe[ta], hbo)
    C[2::4] = vconv_mat(xe[tb], hae)
    C[3::4] = vconv_mat(xe[ta], hbe)
    return C


def build_statics():
    """CT [128, 1024] bf16 = [C0T | C1T | C0sT | C1sT], s = 1/sqrt(2)."""
    C0 = _colifilt_matrix(G0B, G0A)
    C1 = _colifilt_matrix(G1B, G1A)
    s = 1.0 / np.sqrt(2.0)
    CT = np.concatenate([C0.T, C1.T, (s * C0).T, (s * C1).T], axis=1)
    return np.ascontiguousarray(CT.astype(np.float32).astype(NPBF16))


# ---------------- device kernel ----------------


def build_kernel(n_ch=64, G=8, n_cores=8):
    nc = bacc.Bacc("TRN2", target_bir_lowering=False, debug=False,
                   num_devices=n_cores)
    YLT = nc.dram_tensor("YLT", [128, n_ch, 128], BF16, kind="ExternalInput").ap()
    QA = nc.dram_tensor("QA", [3, 128, n_ch, 128], BF16, kind="ExternalInput").ap()
    QB = nc.dram_tensor("QB", [3, 128, n_ch, 128], BF16, kind="ExternalInput").ap()
    CTD = nc.dram_tensor("CT", [128, 1024], BF16, kind="ExternalInput").ap()
    OUT = nc.dram_tensor("Y", [n_ch, 256, 256], BF16, kind="ExternalOutput").ap()

    # group schedule: a few tiny starter groups so the first matmuls fire
    # as soon as ~2 slices of input have landed, then full-size groups
    groups = [(0, 2), (2, 2), (4, 4)]
    g0 = 8
    while g0 < n_ch:
        groups.append((g0, G))
        g0 += G
    assert sum(gl for _, gl in groups) == n_ch

    with tile.TileContext(nc) as tc:
        with (
            tc.tile_pool(name="const", bufs=1) as const,
            tc.tile_pool(name="inp", bufs=2) as inp,
            tc.tile_pool(name="quad", bufs=2) as quad,
            tc.tile_pool(name="zt", bufs=4) as ztp,
            tc.tile_pool(name="yout", bufs=2) as yp,
            tc.tile_pool(name="psz", bufs=3, space="PSUM") as pp,
            tc.tile_pool(name="psy", bufs=3, space="PSUM") as ppy,
        ):
            ct = const.tile([128, 1024], BF16)
            nc.sync.dma_start(ct[:], CTD[:])
            C0T = ct[:, 0:256]
            C1T = ct[:, 256:512]
            C0sT = ct[:, 512:768]
            C1sT = ct[:, 768:1024]

            def load_group(g0, Gl):
                TL = inp.tile([128, Gl * 128], BF16, tag=f"TL{Gl}")
                nc.sync.dma_start(
                    TL.rearrange("p (g c) -> p g c", g=Gl),
                    YLT[:, g0:g0 + Gl],
                )
                qa = inp.tile([128, 3 * Gl * 128], BF16, tag=f"qa{Gl}")
                qb = inp.tile([128, 3 * Gl * 128], BF16, tag=f"qb{Gl}")
                nc.sync.dma_start(
                    qa.rearrange("p (q g c) -> p q g c", q=3, g=Gl),
                    QA[:, :, g0:g0 + Gl].rearrange("q p g c -> p q g c"),
                )
                nc.sync.dma_start(
                    qb.rearrange("p (q g c) -> p q g c", q=3, g=Gl),
                    QB[:, :, g0:g0 + Gl].rearrange("q p g c -> p q g c"),
                )
                return TL, qa, qb, Gl

            def prep_group(state):
                TL, qa, qb, Gl = state
                QD = quad.tile([128, 3 * Gl * 128], BF16, tag=f"qd{Gl}")
                nc.vector.tensor_add(QD[:], qa[:], qb[:])
                return TL, QD, Gl

            def process_group(g0, state):
                TL, QD, Gl = state
                qv = QD.rearrange("p (q g c) -> p q g c", q=3, g=Gl)
                YB = yp.tile([128, Gl * 512], BF16, tag=f"yb{Gl}")
                for ci in range(Gl):
                    cs = slice(ci * 128, (ci + 1) * 128)
                    zp = pp.tile([128, 512], F32, tag="zp")
                    nc.tensor.matmul(zp[:, 0:256], TL[:, cs], C0T,
                                     start=True, stop=False, skip_group_check=True)
                    nc.tensor.matmul(zp[:, 0:256], qv[:, 1, ci], C1sT,
                                     start=False, stop=False, skip_group_check=True)
                    nc.tensor.matmul(zp[:, 256:512], qv[:, 0, ci], C0sT,
                                     start=False, stop=False, skip_group_check=True)
                    nc.tensor.matmul(zp[:, 256:512], qv[:, 2, ci], C1sT,
                                     start=False, stop=True, skip_group_check=True)

                    zs = ztp.tile([128, 512], BF16, tag="zs")
                    nc.scalar.copy(zs[:], zp[:])

                    op = ppy.tile([128, 512], F32, tag="op")
                    nc.tensor.matmul(op[:, 0:256], zs[:, 0:128], C0T,
                                     start=True, stop=False, skip_group_check=True)
                    nc.tensor.matmul(op[:, 0:256], zs[:, 256:384], C1T,
                                     start=False, stop=False, skip_group_check=True)
                    nc.tensor.matmul(op[:, 256:512], zs[:, 128:256], C0T,
                                     start=False, stop=False, skip_group_check=True)
                    nc.tensor.matmul(op[:, 256:512], zs[:, 384:512], C1T,
                                     start=False, stop=True, skip_group_check=True)

                    nc.vector.tensor_copy(YB[:, ci * 512:(ci + 1) * 512], op[:])

                nc.sync.dma_start(
                    OUT[g0:g0 + Gl].rearrange("g (it p) w -> p g it w", it=2),
                    YB.rearrange("p (g it w) -> p g it w", g=Gl, it=2),
                )

            # software pipeline: load + prep g+1 before computing g
            state = prep_group(load_group(*groups[0]))
            for idx, (g0, Gl) in enumerate(groups):
                if idx + 1 < len(groups):
                    nxt = prep_group(load_group(*groups[idx + 1]))
                    process_group(g0, state)
                    state = nxt
                else:
                    process_group(g0, state)

    nc.compile()
    return nc


# ---------------- host wrapper: shard, run on 8 cores, gather ----------------

_CACHED = {}


def _get_compiled():
    if "nc" not in _CACHED:
        _CACHED["nc"] = build_kernel(n_ch=64, G=8, n_cores=8)
        _CACHED["ct"] = build_statics()
    return _CACHED["nc"], _CACHED["ct"]


def _make_in_maps(Yl, Yhr, Yhi, CT):
    """Per-core input packing (pure layout: transpose/interleave/sign)."""
    B = Yl.shape[0]
    # YLT: [B, 128(r), C, 128(c)]
    YLT = np.ascontiguousarray(
        Yl.transpose(0, 2, 1, 3)).astype(NPBF16)
    QA = np.zeros((B, 3, 128, 64, 128), dtype=np.float32)
    QB = np.zeros((B, 3, 128, 64, 128), dtype=np.float32)
    for q, (b1, b2) in enumerate(QUAD_BANDS):
        r1 = Yhr[:, :, b1].transpose(0, 2, 1, 3)   # [B, h, C, w]
        i1 = Yhi[:, :, b1].transpose(0, 2, 1, 3)
        r2 = Yhr[:, :, b2].transpose(0, 2, 1, 3)
        i2 = Yhi[:, :, b2].transpose(0, 2, 1, 3)
        QA[:, q, 0::2, :, 0::2] = r1
        QA[:, q, 0::2, :, 1::2] = i1
        QA[:, q, 1::2, :, 0::2] = i1
        QA[:, q, 1::2, :, 1::2] = -r1
        QB[:, q, 0::2, :, 0::2] = r2
        QB[:, q, 0::2, :, 1::2] = i2
        QB[:, q, 1::2, :, 0::2] = -i2
        QB[:, q, 1::2, :, 1::2] = r2
    QA = QA.astype(NPBF16)
    QB = QB.astype(NPBF16)
    in_maps = []
    for b in range(B):
        in_maps.append({
            "YLT": np.ascontiguousarray(YLT[b]),
            "QA": np.ascontiguousarray(QA[b]),
            "QB": np.ascontiguousarray(QB[b]),
            "CT": CT,
        })
    return in_maps


def kernel(Yl, Yhr, Yhi):
    """Inverse DTCWT (qshift) level. Yl (8,64,128,128) f32,
    Yhr/Yhi (8,64,6,64,64) f32 -> (8,64,256,256) f32.
    Data-parallel over the batch dim: one batch element per NeuronCore."""
    from concourse.bass_utils import run_bass_kernel_spmd

    Yl = np.asarray(Yl, dtype=np.float32)
    Yhr = np.asarray(Yhr, dtype=np.float32)
    Yhi = np.asarray(Yhi, dtype=np.float32)
    B = Yl.shape[0]
    assert B == 8, f"expected batch 8, got {B}"

    nc, CT = _get_compiled()
    in_maps = _make_in_maps(Yl, Yhr, Yhi, CT)
    res = run_bass_kernel_spmd(nc, in_maps, core_ids=list(range(B)))
    out = np.stack([np.asarray(res.results[b]["Y"]) for b in range(B)])
    return out.astype(np.float32)


# revision 9
# speedup vs baseline: 1.1849x; 1.0643x over previous
"""DTCWT inverse (qshift, single level) as a Bass/Tile kernel for TRN2.

Factorization (column filter first):
    out = (C0·Yl + C1·lh)·C0^T + (C0·hl + C1·hh)·C1^T
with C0/C1 the 256x128 banded synthesis (colifilt) matrices; the rowifilt
matrices are identical, so the same SBUF statics serve both stages.

Per slice (128x128 images), two PE stages of 4 matmuls each, all with
256 moving rows:
    stage 1:  z^T[c, i] accumulated in psum [128c, 512] = [z1^T | z2^T]
              A = image (stationary, natural layout), B = C*T (moving)
    stage 2:  out[i-tile] = zs-half^T @ C*T -> psum [128i, 512]
              A = zs[:, tile] (stationary), B = C0T/C1T (moving)

Everything is bf16 (inputs, statics, mid-stage, output) with f32 PSUM
accumulation; validated rel err ~7.8e-3 vs the f32 reference (gate 2e-2).

c2q quadrants: quad = QA + QB where QA/QB are host-packed per-band images
(column-interleaved, signs baked in); one fused DVE add per group. The
1/sqrt(2) c2q scale is folded into the C0s/C1s statics used by stage 1
for the quadrant terms.

All per-slice inputs live in ONE packed dram tensor IN[p, slice, 7, 128]
(Yl row | 3x QA | 3x QB) so each group needs a single dma_start (the
issuing engine pays ~650ns per dma_start, so fewer is faster), with
>=1.75KB contiguous per-partition lines. Output stores go out per
half-group from the gpsimd queue; small starter/final groups shorten
pipeline fill and drain.
"""
import numpy as np
import ml_dtypes

import concourse.bacc as bacc
import concourse.tile as tile
from concourse import mybir

F32 = mybir.dt.float32
BF16 = mybir.dt.bfloat16
NPBF16 = ml_dtypes.bfloat16

QUAD_BANDS = ((2, 3), (0, 5), (1, 4))   # hl, lh, hh

# ---------------- host-side static matrix construction ----------------

_H0A = np.array([0.0351638365171441, 0.0, -0.0883294244510729,
                 0.233890320607236, 0.760272369066126, 0.587518297723561,
                 0.0, -0.114301837144249, 0.0, 0.0], dtype=np.float64)
_H0B = _H0A[::-1].copy()
_ALT = (-1.0) ** np.arange(10)
_H1A = _H0B * _ALT
_H1B = _H1A[::-1].copy()
G0A, G0B, G1A, G1B = _H0B, _H0A, _H1B, _H1A


def _reflect(x, minx, maxx):
    x = np.asarray(x, dtype=np.float64)
    rng = maxx - minx
    rng2 = 2.0 * rng
    mod = np.fmod(x - minx, rng2)
    normed = np.where(mod < 0, mod + rng2, mod)
    return (np.where(normed >= rng, rng2 - normed, normed) + minx).astype(np.int64)


def _colifilt_matrix(ha, hb, r=128):
    """C (2r x r) with colifilt(X) = C @ X."""
    m = ha.shape[0]
    m2 = m // 2
    xe = _reflect(np.arange(-m2, r + m2), -0.5, r - 0.5)
    t = np.arange(2, r + m - 1, 2)
    if float(np.sum(ha * hb)) > 0:
        ta, tb = t, t - 1
    else:
        ta, tb = t - 1, t
    r2 = r // 2
    hao, hae = ha[0::2], ha[1::2]
    hbo, hbe = hb[0::2], hb[1::2]

    def vconv_mat(sel_idx, h):
        hf = h[::-1]
        M = np.zeros((r2, r), dtype=np.float64)
        for i in range(r2):
            for k in range(m2):
                M[i, sel_idx[i + k]] += hf[k]
        return M

    C = np.zeros((2 * r, r), dtype=np.float64)
    C[0::4] = vconv_mat(xe[tb], hao)
    C[1::4] = vconv_mat(xe[ta], hbo)
    C[2::4] = vconv_mat(xe[tb], hae)
    C[3::4] = vconv_mat(xe[ta], hbe)
    return C


def build_statics():
    """CT [128, 1024] bf16 = [C0T | C1T | C0sT | C1sT], s = 1/sqrt(2)."""
    C0 = _colifilt_matrix(G0B, G0A)
    C1 = _colifilt_matrix(G1B, G1A)
    s = 1.0 / np.sqrt(2.0)
    CT = np.concatenate([C0.T, C1.T, (s * C0).T, (s * C1).T], axis=1)
    return np.ascontiguousarray(CT.astype(np.float32).astype(NPBF16))


# ---------------- device kernel ----------------


def build_kernel(n_ch=64, G=8, n_cores=8):
    nc = bacc.Bacc("TRN2", target_bir_lowering=False, debug=False,
                   num_devices=n_cores)
    IN = nc.dram_tensor("IN", [128, n_ch, 7, 128], BF16, kind="ExternalInput").ap()
    CTD = nc.dram_tensor("CT", [128, 1024], BF16, kind="ExternalInput").ap()
    OUT = nc.dram_tensor("Y", [n_ch, 256, 256], BF16, kind="ExternalOutput").ap()

    # group schedule: tiny starter groups so the first matmuls fire early,
    # small final groups so the output-store drain tail is short
    groups = [(0, 2), (2, 2), (4, 4)]
    g0 = 8
    while g0 < n_ch - G:
        groups.append((g0, G))
        g0 += G
    groups += [(g0, 4), (g0 + 4, 2), (g0 + 6, 2)]
    assert sum(gl for _, gl in groups) == n_ch
    assert all(gl in (2, 4, 8) for _, gl in groups)

    with tile.TileContext(nc) as tc:
        with (
            tc.tile_pool(name="const", bufs=1) as const,
            tc.tile_pool(name="inp", bufs=3) as inp,
            tc.tile_pool(name="quad", bufs=3) as quad,
            tc.tile_pool(name="zt", bufs=4) as ztp,
            tc.tile_pool(name="yout", bufs=3) as yp,
            tc.tile_pool(name="psz", bufs=3, space="PSUM") as pp,
            tc.tile_pool(name="psy", bufs=3, space="PSUM") as ppy,
        ):
            ct = const.tile([128, 1024], BF16)
            nc.sync.dma_start(ct[:], CTD[:])
            C0T = ct[:, 0:256]
            C1T = ct[:, 256:512]
            C0sT = ct[:, 512:768]
            C1sT = ct[:, 768:1024]

            def load_group(g0, Gl):
                tin = inp.tile([128, Gl * 7 * 128], BF16, tag=f"in{Gl}")
                nc.sync.dma_start(
                    tin.rearrange("p (g k c) -> p g k c", g=Gl, k=7),
                    IN[:, g0:g0 + Gl],
                )
                return tin, Gl

            def prep_group(state):
                tin, Gl = state
                inv = tin.rearrange("p (g k c) -> p g k c", g=Gl, k=7)
                QD = quad.tile([128, 3 * Gl * 128], BF16, tag=f"qd{Gl}")
                nc.vector.tensor_add(
                    QD.rearrange("p (g q c) -> p g q c", g=Gl, q=3),
                    inv[:, :, 1:4], inv[:, :, 4:7])
                return tin, QD, Gl

            def process_group(g0, state):
                tin, QD, Gl = state
                inv = tin.rearrange("p (g k c) -> p g k c", g=Gl, k=7)
                qv = QD.rearrange("p (g q c) -> p g q c", g=Gl, q=3)
                half = min(Gl, 4)
                for h0 in range(0, Gl, half):
                    YB = yp.tile([128, half * 512], BF16, tag=f"yb{half}")
                    for j in range(half):
                        ci = h0 + j
                        zp = pp.tile([128, 512], F32, tag="zp")
                        nc.tensor.matmul(zp[:, 0:256], inv[:, ci, 0], C0T,
                                         start=True, stop=False, skip_group_check=True)
                        nc.tensor.matmul(zp[:, 0:256], qv[:, ci, 1], C1sT,
                                         start=False, stop=False, skip_group_check=True)
                        nc.tensor.matmul(zp[:, 256:512], qv[:, ci, 0], C0sT,
                                         start=False, stop=False, skip_group_check=True)
                        nc.tensor.matmul(zp[:, 256:512], qv[:, ci, 2], C1sT,
                                         start=False, stop=True, skip_group_check=True)

                        zs = ztp.tile([128, 512], BF16, tag="zs")
                        nc.scalar.copy(zs[:], zp[:])

                        op = ppy.tile([128, 512], F32, tag="op")
                        nc.tensor.matmul(op[:, 0:256], zs[:, 0:128], C0T,
                                         start=True, stop=False, skip_group_check=True)
                        nc.tensor.matmul(op[:, 0:256], zs[:, 256:384], C1T,
                                         start=False, stop=False, skip_group_check=True)
                        nc.tensor.matmul(op[:, 256:512], zs[:, 128:256], C0T,
                                         start=False, stop=False, skip_group_check=True)
                        nc.tensor.matmul(op[:, 256:512], zs[:, 384:512], C1T,
                                         start=False, stop=True, skip_group_check=True)

                        nc.vector.tensor_copy(
                            YB[:, j * 512:(j + 1) * 512], op[:])

                    nc.gpsimd.dma_start(
                        OUT[g0 + h0:g0 + h0 + half].rearrange(
                            "g (it p) w -> p g it w", it=2),
                        YB.rearrange("p (g it w) -> p g it w", g=half, it=2),
                    )

            # software pipeline: load + prep g+1 before computing g
            state = prep_group(load_group(*groups[0]))
            for idx, (g0, Gl) in enumerate(groups):
                if idx + 1 < len(groups):
                    nxt = prep_group(load_group(*groups[idx + 1]))
                    process_group(g0, state)
                    state = nxt
                else:
                    process_group(g0, state)

    nc.compile()
    return nc


# ---------------- host wrapper: shard, run on 8 cores, gather ----------------

_CACHED = {}


def _get_compiled():
    if "nc" not in _CACHED:
        _CACHED["nc"] = build_kernel(n_ch=64, G=8, n_cores=8)
        _CACHED["ct"] = build_statics()
    return _CACHED["nc"], _CACHED["ct"]


def _make_in_maps(Yl, Yhr, Yhi, CT):
    """Per-core input packing (pure layout: transpose/interleave/sign).
    IN[b] = [128(p), C, 7, 128]: k=0 Yl row p, k=1..3 QA quads, k=4..6 QB."""
    B = Yl.shape[0]
    IN = np.zeros((B, 128, 64, 7, 128), dtype=np.float32)
    IN[:, :, :, 0, :] = Yl.transpose(0, 2, 1, 3)
    for q, (b1, b2) in enumerate(QUAD_BANDS):
        r1 = Yhr[:, :, b1].transpose(0, 2, 1, 3)   # [B, h, C, w]
        i1 = Yhi[:, :, b1].transpose(0, 2, 1, 3)
        r2 = Yhr[:, :, b2].transpose(0, 2, 1, 3)
        i2 = Yhi[:, :, b2].transpose(0, 2, 1, 3)
        IN[:, 0::2, :, 1 + q, 0::2] = r1
        IN[:, 0::2, :, 1 + q, 1::2] = i1
        IN[:, 1::2, :, 1 + q, 0::2] = i1
        IN[:, 1::2, :, 1 + q, 1::2] = -r1
        IN[:, 0::2, :, 4 + q, 0::2] = r2
        IN[:, 0::2, :, 4 + q, 1::2] = i2
        IN[:, 1::2, :, 4 + q, 0::2] = -i2
        IN[:, 1::2, :, 4 + q, 1::2] = r2
    IN = IN.astype(NPBF16)
    return [{"IN": np.ascontiguousarray(IN[b]), "CT": CT} for b in range(B)]


def kernel(Yl, Yhr, Yhi):
    """Inverse DTCWT (qshift) level. Yl (8,64,128,128) f32,
    Yhr/Yhi (8,64,6,64,64) f32 -> (8,64,256,256) f32.
    Data-parallel over the batch dim: one batch element per NeuronCore."""
    from concourse.bass_utils import run_bass_kernel_spmd

    Yl = np.asarray(Yl, dtype=np.float32)
    Yhr = np.asarray(Yhr, dtype=np.float32)
    Yhi = np.asarray(Yhi, dtype=np.float32)
    B = Yl.shape[0]
    assert B == 8, f"expected batch 8, got {B}"

    nc, CT = _get_compiled()
    in_maps = _make_in_maps(Yl, Yhr, Yhi, CT)
    res = run_bass_kernel_spmd(nc, in_maps, core_ids=list(range(B)))
    out = np.stack([np.asarray(res.results[b]["Y"]) for b in range(B)])
    return out.astype(np.float32)
